# revision 15
# baseline (speedup 1.0000x reference)
"""ComplexDenseSO2 Trainium2 kernel (v3).

Computes out = (X @ conj(B)^T * w) @ B for complex X [64, 32400],
B [2048, 32400], w [2048], given as separate re/im fp32 planes.

Strategy (tensor-parallel over D across 8 cores):
  - Fold w into the first-matmul operand on the host:
    M = diag(w) @ conj(B), so mm1 output IS Y = X @ M^T.
  - Pad D 32400 -> 32768; core c owns d-slice [c*4096, (c+1)*4096).
  - mm1 merges re/im into ONE PSUM accumulator using two stationary
    variants S_A = (Xr|Xi), S_B = (-Xi|Xr): rows 0:64 = Yr, 64:128 = Yi.
    k is processed in 2 HALVES of 1024 (2 PSUM banks each) so each
    half's 256KB AllReduce overlaps the other half's compute and the
    start of mm2.
  - Per half: PSUM -> SBUF f32, PE-transpose to [k, j], cast fp16, one
    contiguous 256KB DMA to a [128, 1024]-layout DRAM bounce (the AR is
    elementwise, so the SBUF-native layout avoids small descriptors),
    AllReduce(add) over the 8 cores.
  - mm2: k-outer with all 8 PSUM banks holding the full [128, 4096]
    f32 output; stationaries ytA = Y^T, ytB = (-Yi^T|Yr^T) built from
    the AR outputs; rhs B tiles streamed as 1MB fully-contiguous DMAs.
  - DMA engine split: input streams (X, M, B) issue on nc.sync;
    AR-coupled transfers (arin writes, arout reads) issue on
    nc.scalar with explicit ordering deps so the scheduler cannot
    park an AR-output read in front of a later AR-input write.
  - Host pre-lays out every tensor so each big DMA is >=1MB with
    >=8KB contiguous bytes per partition line.
  - fp16 operands use power-of-2 prescales (M*1024, B*256) to stay
    clear of fp16 subnormals; the epilogue descales by 2^-18.
"""

import sys

if "/opt/trn_rl_repo" not in sys.path:
    sys.path.insert(0, "/opt/trn_rl_repo")

import numpy as np

B_, K, D = 64, 2048, 32400
NCORES = 8
DP = 32768
DL = DP // NCORES  # 4096
NDT = DL // 128    # 32 d-tiles
NQ = 2             # k-halves
KW = K // NQ       # 1024 k per half
NKB = K // 128     # 16 k-blocks
MCH = 4            # d-tiles per M DMA chunk (1MB per component)

SCALE_M = 1024.0
SCALE_B = 256.0

_nc_cache = {}


def build_nc(n_cores=NCORES):
    import concourse.mybir as mybir
    from concourse import bacc
    import concourse.tile as tile
    from concourse.masks import make_identity
    from concourse.tile_rust import add_dep_helper

    fp = mybir.dt.float16
    f32 = mybir.dt.float32

    nc = bacc.Bacc(
        trn_type="TRN2",
        target_bir_lowering=False,
        debug=False,
        num_devices=n_cores,
    )
    # xa[p, dt*128+j] = S_A[dt*128+p, j]; S_A = (Xr|Xi) along j
    xa = nc.dram_tensor("xa", [128, DL], fp, kind="ExternalInput")
    xb = nc.dram_tensor("xb", [128, DL], fp, kind="ExternalInput")
    # mrh[p, (q*NDT+dt)*KW + kk] = Mr[q*KW+kk, lo+dt*128+p]
    mrh = nc.dram_tensor("mrh", [128, NQ * NDT * KW], fp, kind="ExternalInput")
    mih = nc.dram_tensor("mih", [128, NQ * NDT * KW], fp, kind="ExternalInput")
    # bnr[p, kb*DL + dd] = Br[kb*128+p, lo+dd] * SCALE_B
    bnr = nc.dram_tensor("bnr", [128, NKB * DL], fp, kind="ExternalInput")
    bni = nc.dram_tensor("bni", [128, NKB * DL], fp, kind="ExternalInput")
    out = nc.dram_tensor("out", [128, DL], f32, kind="ExternalOutput")

    with tile.TileContext(nc) as tc:
        with (
            tc.tile_pool(name="sb", bufs=1) as sb,
            tc.tile_pool(name="sbx", bufs=1) as sbx,
            tc.tile_pool(name="ps", bufs=1, space="PSUM") as ps,
            tc.tile_pool(name="dram", bufs=1, space="DRAM") as dram,
        ):
            ident = sbx.tile([128, 128], f32, tag="ident")
            make_identity(nc, ident)

            xa_s = sbx.tile([128, DL], fp, tag="xa_s", name="xa_s")
            nc.sync.dma_start(out=xa_s, in_=xa.ap())
            xb_s = sbx.tile([128, DL], fp, tag="xb_s", name="xb_s")
            nc.sync.dma_start(out=xb_s, in_=xb.ap())

            # Warm up the collectives path: the first AllReduce on the cc
            # stream pays ~11us of ncfw wakeup latency that later ones do
            # not. Fire a 512B dummy AllReduce (one column of the identity
            # tile) at kernel start so the real ARs trigger with warm-path
            # latency.
            dmy_in = dram.tile([128, 1], f32, tag="dmy_in", name="dmy_in")
            dmy_out = dram.tile(
                [128, 1], f32, tag="dmy_out", name="dmy_out", addr_space="Shared"
            )
            dmy_w = nc.scalar.dma_start(out=dmy_in, in_=ident[:, 0:1])
            nc.gpsimd.collective_compute(
                "AllReduce",
                mybir.AluOpType.add,
                ins=[dmy_in.opt()],
                outs=[dmy_out.opt()],
                replica_groups=[list(range(n_cores))],
            )

            # AR payload layout is the SBUF-native [128, KW] (the AllReduce
            # is elementwise, so any consistent layout works) -- this keeps
            # the arin write and arout read fully contiguous per partition.
            arin = []
            arout = []
            for q in range(NQ):
                ai = dram.tile([128, KW], fp, tag=f"arin{q}", name=f"arin{q}")
                ao = dram.tile(
                    [128, KW], fp, tag=f"arout{q}", name=f"arout{q}",
                    addr_space="Shared",
                )
                arin.append(ai)
                arout.append(ao)

            # ---------------- mm1: two k-halves ----------------
            scalar_dmas = [dmy_w]  # for explicit ordering on the scalar queue
            for q in range(NQ):
                acc0 = ps.tile([128, 512], f32, tag=f"p{2 * q}", name="acc0", bufs=1)
                acc1 = ps.tile([128, 512], f32, tag=f"p{2 * q + 1}", name="acc1", bufs=1)
                for g in range(NDT // MCH):
                    base = (q * NDT + g * MCH) * KW
                    mr_t = sb.tile([128, MCH * KW], fp, tag="mr", name="mr", bufs=4)
                    nc.sync.dma_start(out=mr_t, in_=mrh[:, base : base + MCH * KW])
                    mi_t = sb.tile([128, MCH * KW], fp, tag="mi", name="mi", bufs=4)
                    nc.sync.dma_start(out=mi_t, in_=mih[:, base : base + MCH * KW])
                    for t in range(MCH):
                        dt = g * MCH + t
                        st, sp = dt == 0, dt == NDT - 1
                        xs_a = xa_s[:, dt * 128 : (dt + 1) * 128]
                        xs_b = xb_s[:, dt * 128 : (dt + 1) * 128]
                        ts = t * KW
                        nc.tensor.matmul(acc0, lhsT=xs_a, rhs=mr_t[:, ts : ts + 512], start=st, stop=False)
                        nc.tensor.matmul(acc1, lhsT=xs_a, rhs=mr_t[:, ts + 512 : ts + 1024], start=st, stop=False)
                        nc.tensor.matmul(acc0, lhsT=xs_b, rhs=mi_t[:, ts : ts + 512], start=False, stop=sp)
                        nc.tensor.matmul(acc1, lhsT=xs_b, rhs=mi_t[:, ts + 512 : ts + 1024], start=False, stop=sp)

                # Evacuate Y[j, half], transpose to [k, j], ship to AR.
                pscr = sb.tile([128, KW], f32, tag="pscr", name="pscr", bufs=1)
                nc.vector.tensor_copy(pscr[:, 0:512], acc0)
                nc.scalar.copy(pscr[:, 512:1024], acc1)
                yt_sb = sb.tile([128, KW], fp, tag="yt_sb", name="yt_sb", bufs=1)
                for b in range(KW // 128):
                    tp = ps.tile([128, 128], f32, tag=f"p{4 + (b % 2)}", name="tp", bufs=1)
                    nc.tensor.transpose(tp, pscr[:, b * 128 : (b + 1) * 128], ident)
                    nc.vector.tensor_copy(yt_sb[:, b * 128 : (b + 1) * 128], tp)
                w_inst = nc.scalar.dma_start(out=arin[q], in_=yt_sb)
                scalar_dmas.append(w_inst)
                nc.gpsimd.collective_compute(
                    "AllReduce",
                    mybir.AluOpType.add,
                    ins=[arin[q].opt()],
                    outs=[arout[q].opt()],
                    replica_groups=[list(range(n_cores))],
                )

            # ---------------- mm2 stationaries from AR outputs ----------------
            ytA = []
            ytB = []
            for q in range(NQ):
                a_t = sbx.tile([128, KW], fp, tag=f"ytA{q}", name=f"ytA{q}")
                r_inst = nc.scalar.dma_start(out=a_t, in_=arout[q])
                scalar_dmas.append(r_inst)
                b_t = sbx.tile([128, KW], fp, tag=f"ytB{q}", name=f"ytB{q}")
                for b in range(KW // 128):
                    o = b * 128
                    nc.vector.tensor_scalar_mul(b_t[:, o : o + 64], a_t[:, o + 64 : o + 128], -1.0)
                    nc.vector.tensor_copy(b_t[:, o + 64 : o + 128], a_t[:, o : o + 64])
                ytA.append(a_t)
                ytB.append(b_t)

            # Pin the scalar-queue order: every arout read sits after every
            # arin write, and the queue follows emission order. Without this
            # the scheduler can park an AR-output read (waiting on AR k) in
            # front of a later AR-input write, stalling the next AR.
            from_i = None
            for inst in scalar_dmas:
                if from_i is not None:
                    add_dep_helper(inst.ins, from_i.ins, sync=False,
                                   reason="scalar DMA queue order")
                from_i = inst

            # ---------------- mm2: k-outer, 8 PSUM banks ----------------
            pos = []
            for i in range(8):
                po = ps.tile([128, 512], f32, tag=f"p{i}", name=f"po{i}", bufs=1)
                pos.append(po)
            for kb in range(NKB):
                q, b = kb // (KW // 128), kb % (KW // 128)
                br_t = sb.tile([128, DL], fp, tag="br", name="br", bufs=5)
                nc.sync.dma_start(out=br_t, in_=bnr[:, kb * DL : (kb + 1) * DL])
                bi_t = sb.tile([128, DL], fp, tag="bi", name="bi", bufs=5)
                nc.sync.dma_start(out=bi_t, in_=bni[:, kb * DL : (kb + 1) * DL])
                ya = ytA[q][:, b * 128 : (b + 1) * 128]
                yb = ytB[q][:, b * 128 : (b + 1) * 128]
                st, sp = kb == 0, kb == NKB - 1
                for i in range(8):
                    nc.tensor.matmul(pos[i], lhsT=ya, rhs=br_t[:, i * 512 : (i + 1) * 512], start=st, stop=False)
                for i in range(8):
                    nc.tensor.matmul(pos[i], lhsT=yb, rhs=bi_t[:, i * 512 : (i + 1) * 512], start=False, stop=sp)

            # Descale by 1/(SCALE_M*SCALE_B) happens on the host during
            # output assembly; here only evacuate PSUM -> SBUF.
            osb = sb.tile([128, DL], f32, tag="osb", name="osb", bufs=1)
            for i in range(8):
                if i % 2 == 0:
                    nc.vector.tensor_copy(osb[:, i * 512 : (i + 1) * 512], pos[i])
                else:
                    nc.scalar.copy(osb[:, i * 512 : (i + 1) * 512], pos[i])
            nc.sync.dma_start(out=out[:, 0 : DL // 2], in_=osb[:, 0 : DL // 2])
            nc.sync.dma_start(out=out[:, DL // 2 : DL], in_=osb[:, DL // 2 : DL])

    nc.compile()
    return nc


def _get_nc(n_cores=NCORES):
    if n_cores not in _nc_cache:
        _nc_cache[n_cores] = build_nc(n_cores)
    return _nc_cache[n_cores]


def _prep_in_maps(X_re, X_im, bases_re, bases_im, weight_re, weight_im):
    cdt = np.float16
    f32 = np.float32
    X_re = np.asarray(X_re, f32)
    X_im = np.asarray(X_im, f32)
    bases_re = np.asarray(bases_re, f32)
    bases_im = np.asarray(bases_im, f32)
    wr = np.asarray(weight_re, f32)[:, None]
    wi = np.asarray(weight_im, f32)[:, None]

    # M = diag(w) @ conj(B): Mr = wr*Br + wi*Bi ; Mi = wi*Br - wr*Bi
    mr = (wr * bases_re + wi * bases_im) * f32(SCALE_M)
    mi = (wi * bases_re - wr * bases_im) * f32(SCALE_M)
    bsr = bases_re * f32(SCALE_B)
    bsi = bases_im * f32(SCALE_B)

    in_maps = []
    for c in range(NCORES):
        lo = c * DL
        hi = min((c + 1) * DL, D)
        n = hi - lo

        xat = np.zeros((DL, 128), f32)
        xbt = np.zeros((DL, 128), f32)
        if n > 0:
            xat[:n, 0:64] = X_re[:, lo:hi].T
            xat[:n, 64:128] = X_im[:, lo:hi].T
            xbt[:n, 0:64] = -X_im[:, lo:hi].T
            xbt[:n, 64:128] = X_re[:, lo:hi].T
        # [DL,128] -> [128, DL] with xa[p, dt*128+j] = xat[dt*128+p, j]
        xa = xat.reshape(NDT, 128, 128).transpose(1, 0, 2).reshape(128, DL).astype(cdt)
        xb = xbt.reshape(NDT, 128, 128).transpose(1, 0, 2).reshape(128, DL).astype(cdt)

        def m_layout(m):
            mp = np.zeros((K, DL), f32)
            if n > 0:
                mp[:, :n] = m[:, lo:hi]
            # mrh[p, (q*NDT+dt)*KW + kk] = mp[q*KW+kk, dt*128+p]
            t = mp.reshape(NQ, KW, NDT, 128)
            return t.transpose(3, 0, 2, 1).reshape(128, NQ * NDT * KW).astype(cdt)

        def b_layout(bm):
            bp = np.zeros((K, DL), f32)
            if n > 0:
                bp[:, :n] = bm[:, lo:hi]
            # bnr[p, kb*DL + dd] = bp[kb*128+p, dd]
            t = bp.reshape(NKB, 128, DL)
            return t.transpose(1, 0, 2).reshape(128, NKB * DL).astype(cdt)

        in_maps.append({
            "xa": xa,
            "xb": xb,
            "mrh": m_layout(mr),
            "mih": m_layout(mi),
            "bnr": b_layout(bsr),
            "bni": b_layout(bsi),
        })
    return in_maps


def run(inputs, trace=False, trace_kwargs=None):
    """Returns (full complex64 output [64, 32400], BassKernelResults)."""
    from concourse.bass_utils import run_bass_kernel_spmd

    in_maps = _prep_in_maps(**inputs)
    nc = _get_nc()
    res = run_bass_kernel_spmd(
        nc,
        in_maps,
        core_ids=list(range(NCORES)),
        trace=trace,
        **(trace_kwargs or {}),
    )
    dsc = np.float32(1.0 / (SCALE_M * SCALE_B))
    parts = []
    for c in range(NCORES):
        o = res.results[c]["out"]
        parts.append(o[0:64, :] + 1j * o[64:128, :].astype(np.complex64))
    full = (np.concatenate(parts, axis=1)[:, :D] * dsc).astype(np.complex64)
    return full, res


def kernel(**inputs) -> np.ndarray:
    out, _ = run(inputs, trace=False)
    return out


# revision 16
# speedup vs baseline: 1.0407x; 1.0407x over previous
"""ComplexDenseSO2 Trainium2 kernel (v3).

Computes out = (X @ conj(B)^T * w) @ B for complex X [64, 32400],
B [2048, 32400], w [2048], given as separate re/im fp32 planes.

Strategy (tensor-parallel over D across 8 cores):
  - Fold w into the first-matmul operand on the host:
    M = diag(w) @ conj(B), so mm1 output IS Y = X @ M^T.
  - Pad D 32400 -> 32768; core c owns d-slice [c*4096, (c+1)*4096).
  - mm1 merges re/im into ONE PSUM accumulator using two stationary
    variants S_A = (Xr|Xi), S_B = (-Xi|Xr): rows 0:64 = Yr, 64:128 = Yi.
    k is processed in 2 HALVES of 1024 (2 PSUM banks each) so each
    half's 256KB AllReduce overlaps the other half's compute and the
    start of mm2.
  - Per half: PSUM -> SBUF f32, PE-transpose to [k, j], cast fp16, one
    contiguous 256KB DMA to a [128, 1024]-layout DRAM bounce (the AR is
    elementwise, so the SBUF-native layout avoids small descriptors),
    AllReduce(add) over the 8 cores.
  - mm2: k-outer with all 8 PSUM banks holding the full [128, 4096]
    f32 output; stationaries ytA = Y^T, ytB = (-Yi^T|Yr^T) built from
    the AR outputs; rhs B tiles streamed as 1MB fully-contiguous DMAs.
  - DMA engine split: input streams (X, M, B) issue on nc.sync;
    AR-coupled transfers (arin writes, arout reads) issue on
    nc.scalar with explicit ordering deps so the scheduler cannot
    park an AR-output read in front of a later AR-input write.
  - Host pre-lays out every tensor so each big DMA is >=1MB with
    >=8KB contiguous bytes per partition line.
  - fp16 operands use power-of-2 prescales (M*1024, B*256) to stay
    clear of fp16 subnormals; the epilogue descales by 2^-18.
"""

import sys

if "/opt/trn_rl_repo" not in sys.path:
    sys.path.insert(0, "/opt/trn_rl_repo")

import numpy as np

B_, K, D = 64, 2048, 32400
NCORES = 8
DP = 32768
DL = DP // NCORES  # 4096
NDT = DL // 128    # 32 d-tiles
NQ = 2             # k-halves
KW = K // NQ       # 1024 k per half
NKB = K // 128     # 16 k-blocks
MCH = 4            # d-tiles per M DMA chunk (1MB per component)

SCALE_M = 1024.0
SCALE_B = 256.0

_nc_cache = {}


def build_nc(n_cores=NCORES):
    import concourse.mybir as mybir
    from concourse import bacc
    import concourse.tile as tile
    from concourse.masks import make_identity
    from concourse.tile_rust import add_dep_helper

    fp = mybir.dt.float16
    f32 = mybir.dt.float32

    nc = bacc.Bacc(
        trn_type="TRN2",
        target_bir_lowering=False,
        debug=False,
        num_devices=n_cores,
    )
    # xa[p, dt*128+j] = S_A[dt*128+p, j]; S_A = (Xr|Xi) along j
    xa = nc.dram_tensor("xa", [128, DL], fp, kind="ExternalInput")
    xb = nc.dram_tensor("xb", [128, DL], fp, kind="ExternalInput")
    # mrh[p, (q*NDT+dt)*KW + kk] = Mr[q*KW+kk, lo+dt*128+p]
    mrh = nc.dram_tensor("mrh", [128, NQ * NDT * KW], fp, kind="ExternalInput")
    mih = nc.dram_tensor("mih", [128, NQ * NDT * KW], fp, kind="ExternalInput")
    # bnr[p, kb*DL + dd] = Br[kb*128+p, lo+dd] * SCALE_B
    bnr = nc.dram_tensor("bnr", [128, NKB * DL], fp, kind="ExternalInput")
    bni = nc.dram_tensor("bni", [128, NKB * DL], fp, kind="ExternalInput")
    out = nc.dram_tensor("out", [128, DL], f32, kind="ExternalOutput")

    with tile.TileContext(nc) as tc:
        with (
            tc.tile_pool(name="sb", bufs=1) as sb,
            tc.tile_pool(name="sbx", bufs=1) as sbx,
            tc.tile_pool(name="ps", bufs=1, space="PSUM") as ps,
            tc.tile_pool(name="dram", bufs=1, space="DRAM") as dram,
        ):
            ident = sbx.tile([128, 128], f32, tag="ident")
            make_identity(nc, ident)

            xa_s = sbx.tile([128, DL], fp, tag="xa_s", name="xa_s")
            nc.sync.dma_start(out=xa_s, in_=xa.ap())
            xb_s = sbx.tile([128, DL], fp, tag="xb_s", name="xb_s")
            nc.sync.dma_start(out=xb_s, in_=xb.ap())

            # AR payload layout is the SBUF-native [128, KW] (the AllReduce
            # is elementwise, so any consistent layout works) -- this keeps
            # the arin write and arout read fully contiguous per partition.
            arin = []
            arout = []
            for q in range(NQ):
                ai = dram.tile([128, KW], fp, tag=f"arin{q}", name=f"arin{q}")
                ao = dram.tile(
                    [128, KW], fp, tag=f"arout{q}", name=f"arout{q}",
                    addr_space="Shared",
                )
                arin.append(ai)
                arout.append(ao)

            # ---------------- mm1: two k-halves ----------------
            scalar_dmas = []  # for explicit ordering on the scalar queue
            for q in range(NQ):
                acc0 = ps.tile([128, 512], f32, tag=f"p{2 * q}", name="acc0", bufs=1)
                acc1 = ps.tile([128, 512], f32, tag=f"p{2 * q + 1}", name="acc1", bufs=1)
                for g in range(NDT // MCH):
                    base = (q * NDT + g * MCH) * KW
                    mr_t = sb.tile([128, MCH * KW], fp, tag="mr", name="mr", bufs=4)
                    nc.sync.dma_start(out=mr_t, in_=mrh[:, base : base + MCH * KW])
                    mi_t = sb.tile([128, MCH * KW], fp, tag="mi", name="mi", bufs=4)
                    nc.sync.dma_start(out=mi_t, in_=mih[:, base : base + MCH * KW])
                    for t in range(MCH):
                        dt = g * MCH + t
                        st, sp = dt == 0, dt == NDT - 1
                        xs_a = xa_s[:, dt * 128 : (dt + 1) * 128]
                        xs_b = xb_s[:, dt * 128 : (dt + 1) * 128]
                        ts = t * KW
                        nc.tensor.matmul(acc0, lhsT=xs_a, rhs=mr_t[:, ts : ts + 512], start=st, stop=False)
                        nc.tensor.matmul(acc1, lhsT=xs_a, rhs=mr_t[:, ts + 512 : ts + 1024], start=st, stop=False)
                        nc.tensor.matmul(acc0, lhsT=xs_b, rhs=mi_t[:, ts : ts + 512], start=False, stop=sp)
                        nc.tensor.matmul(acc1, lhsT=xs_b, rhs=mi_t[:, ts + 512 : ts + 1024], start=False, stop=sp)

                # Evacuate Y[j, half], transpose to [k, j], ship to AR.
                pscr = sb.tile([128, KW], f32, tag="pscr", name="pscr", bufs=1)
                nc.vector.tensor_copy(pscr[:, 0:512], acc0)
                nc.scalar.copy(pscr[:, 512:1024], acc1)
                yt_sb = sb.tile([128, KW], fp, tag="yt_sb", name="yt_sb", bufs=1)
                for b in range(KW // 128):
                    tp = ps.tile([128, 128], f32, tag=f"p{4 + (b % 2)}", name="tp", bufs=1)
                    nc.tensor.transpose(tp, pscr[:, b * 128 : (b + 1) * 128], ident)
                    nc.vector.tensor_copy(yt_sb[:, b * 128 : (b + 1) * 128], tp)
                w_inst = nc.scalar.dma_start(out=arin[q], in_=yt_sb)
                scalar_dmas.append(w_inst)
                nc.gpsimd.collective_compute(
                    "AllReduce",
                    mybir.AluOpType.add,
                    ins=[arin[q].opt()],
                    outs=[arout[q].opt()],
                    replica_groups=[list(range(n_cores))],
                )

            # ---------------- mm2 stationaries from AR outputs ----------------
            ytA = []
            ytB = []
            for q in range(NQ):
                a_t = sbx.tile([128, KW], fp, tag=f"ytA{q}", name=f"ytA{q}")
                r_inst = nc.scalar.dma_start(out=a_t, in_=arout[q])
                scalar_dmas.append(r_inst)
                b_t = sbx.tile([128, KW], fp, tag=f"ytB{q}", name=f"ytB{q}")
                for b in range(KW // 128):
                    o = b * 128
                    nc.vector.tensor_scalar_mul(b_t[:, o : o + 64], a_t[:, o + 64 : o + 128], -1.0)
                    nc.vector.tensor_copy(b_t[:, o + 64 : o + 128], a_t[:, o : o + 64])
                ytA.append(a_t)
                ytB.append(b_t)

            # Pin the scalar-queue order: every arout read sits after every
            # arin write, and the queue follows emission order. Without this
            # the scheduler can park an AR-output read (waiting on AR k) in
            # front of a later AR-input write, stalling the next AR.
            from_i = None
            for inst in scalar_dmas:
                if from_i is not None:
                    add_dep_helper(inst.ins, from_i.ins, sync=False,
                                   reason="scalar DMA queue order")
                from_i = inst

            # ---------------- mm2: k-outer, 8 PSUM banks ----------------
            pos = []
            for i in range(8):
                po = ps.tile([128, 512], f32, tag=f"p{i}", name=f"po{i}", bufs=1)
                pos.append(po)
            for kb in range(NKB):
                q, b = kb // (KW // 128), kb % (KW // 128)
                br_t = sb.tile([128, DL], fp, tag="br", name="br", bufs=5)
                nc.sync.dma_start(out=br_t, in_=bnr[:, kb * DL : (kb + 1) * DL])
                bi_t = sb.tile([128, DL], fp, tag="bi", name="bi", bufs=5)
                nc.sync.dma_start(out=bi_t, in_=bni[:, kb * DL : (kb + 1) * DL])
                ya = ytA[q][:, b * 128 : (b + 1) * 128]
                yb = ytB[q][:, b * 128 : (b + 1) * 128]
                st, sp = kb == 0, kb == NKB - 1
                for i in range(8):
                    nc.tensor.matmul(pos[i], lhsT=ya, rhs=br_t[:, i * 512 : (i + 1) * 512], start=st, stop=False)
                for i in range(8):
                    nc.tensor.matmul(pos[i], lhsT=yb, rhs=bi_t[:, i * 512 : (i + 1) * 512], start=False, stop=sp)

            # Descale by 1/(SCALE_M*SCALE_B) happens on the host during
            # output assembly; here only evacuate PSUM -> SBUF.
            osb = sb.tile([128, DL], f32, tag="osb", name="osb", bufs=1)
            for i in range(8):
                if i % 2 == 0:
                    nc.vector.tensor_copy(osb[:, i * 512 : (i + 1) * 512], pos[i])
                else:
                    nc.scalar.copy(osb[:, i * 512 : (i + 1) * 512], pos[i])
            nc.sync.dma_start(out=out[:, 0 : DL // 2], in_=osb[:, 0 : DL // 2])
            nc.sync.dma_start(out=out[:, DL // 2 : DL], in_=osb[:, DL // 2 : DL])

    nc.compile()
    return nc


def _get_nc(n_cores=NCORES):
    if n_cores not in _nc_cache:
        _nc_cache[n_cores] = build_nc(n_cores)
    return _nc_cache[n_cores]


def _prep_in_maps(X_re, X_im, bases_re, bases_im, weight_re, weight_im):
    cdt = np.float16
    f32 = np.float32
    X_re = np.asarray(X_re, f32)
    X_im = np.asarray(X_im, f32)
    bases_re = np.asarray(bases_re, f32)
    bases_im = np.asarray(bases_im, f32)
    wr = np.asarray(weight_re, f32)[:, None]
    wi = np.asarray(weight_im, f32)[:, None]

    # M = diag(w) @ conj(B): Mr = wr*Br + wi*Bi ; Mi = wi*Br - wr*Bi
    mr = (wr * bases_re + wi * bases_im) * f32(SCALE_M)
    mi = (wi * bases_re - wr * bases_im) * f32(SCALE_M)
    bsr = bases_re * f32(SCALE_B)
    bsi = bases_im * f32(SCALE_B)

    in_maps = []
    for c in range(NCORES):
        lo = c * DL
        hi = min((c + 1) * DL, D)
        n = hi - lo

        xat = np.zeros((DL, 128), f32)
        xbt = np.zeros((DL, 128), f32)
        if n > 0:
            xat[:n, 0:64] = X_re[:, lo:hi].T
            xat[:n, 64:128] = X_im[:, lo:hi].T
            xbt[:n, 0:64] = -X_im[:, lo:hi].T
            xbt[:n, 64:128] = X_re[:, lo:hi].T
        # [DL,128] -> [128, DL] with xa[p, dt*128+j] = xat[dt*128+p, j]
        xa = xat.reshape(NDT, 128, 128).transpose(1, 0, 2).reshape(128, DL).astype(cdt)
        xb = xbt.reshape(NDT, 128, 128).transpose(1, 0, 2).reshape(128, DL).astype(cdt)

        def m_layout(m):
            mp = np.zeros((K, DL), f32)
            if n > 0:
                mp[:, :n] = m[:, lo:hi]
            # mrh[p, (q*NDT+dt)*KW + kk] = mp[q*KW+kk, dt*128+p]
            t = mp.reshape(NQ, KW, NDT, 128)
            return t.transpose(3, 0, 2, 1).reshape(128, NQ * NDT * KW).astype(cdt)

        def b_layout(bm):
            bp = np.zeros((K, DL), f32)
            if n > 0:
                bp[:, :n] = bm[:, lo:hi]
            # bnr[p, kb*DL + dd] = bp[kb*128+p, dd]
            t = bp.reshape(NKB, 128, DL)
            return t.transpose(1, 0, 2).reshape(128, NKB * DL).astype(cdt)

        in_maps.append({
            "xa": xa,
            "xb": xb,
            "mrh": m_layout(mr),
            "mih": m_layout(mi),
            "bnr": b_layout(bsr),
            "bni": b_layout(bsi),
        })
    return in_maps


def run(inputs, trace=False, trace_kwargs=None):
    """Returns (full complex64 output [64, 32400], BassKernelResults)."""
    from concourse.bass_utils import run_bass_kernel_spmd

    in_maps = _prep_in_maps(**inputs)
    nc = _get_nc()
    res = run_bass_kernel_spmd(
        nc,
        in_maps,
        core_ids=list(range(NCORES)),
        trace=trace,
        **(trace_kwargs or {}),
    )
    dsc = np.float32(1.0 / (SCALE_M * SCALE_B))
    parts = []
    for c in range(NCORES):
        o = res.results[c]["out"]
        parts.append(o[0:64, :] + 1j * o[64:128, :].astype(np.complex64))
    full = (np.concatenate(parts, axis=1)[:, :D] * dsc).astype(np.complex64)
    return full, res


def kernel(**inputs) -> np.ndarray:
    out, _ = run(inputs, trace=False)
    return out


# revision 17
# speedup vs baseline: 1.0711x; 1.0293x over previous
"""ComplexDenseSO2 Trainium2 kernel (v3).

Computes out = (X @ conj(B)^T * w) @ B for complex X [64, 32400],
B [2048, 32400], w [2048], given as separate re/im fp32 planes.

Strategy (tensor-parallel over D across 8 cores):
  - Fold w into the first-matmul operand on the host:
    M = diag(w) @ conj(B), so mm1 output IS Y = X @ M^T.
  - Pad D 32400 -> 32768; core c owns d-slice [c*4096, (c+1)*4096).
  - mm1 merges re/im into ONE PSUM accumulator using two stationary
    variants S_A = (Xr|Xi), S_B = (-Xi|Xr): rows 0:64 = Yr, 64:128 = Yi.
    k is processed in 2 HALVES of 1024 (2 PSUM banks each) so each
    half's 256KB AllReduce overlaps the other half's compute and the
    start of mm2.
  - Per half: PSUM -> SBUF f32, PE-transpose to [k, j], cast fp16, one
    contiguous 256KB DMA to a [128, 1024]-layout DRAM bounce (the AR is
    elementwise, so the SBUF-native layout avoids small descriptors),
    AllReduce(add) over the 8 cores.
  - mm2: k-outer with all 8 PSUM banks holding the full [128, 4096]
    f32 output; stationaries ytA = Y^T, ytB = (-Yi^T|Yr^T) built from
    the AR outputs; rhs B tiles streamed as 1MB fully-contiguous DMAs.
  - DMA engine split: input streams (X, M, B) issue on nc.sync;
    AR-coupled transfers (arin writes, arout reads) issue on
    nc.scalar with explicit ordering deps so the scheduler cannot
    park an AR-output read in front of a later AR-input write.
  - Host pre-lays out every tensor so each big DMA is >=1MB with
    >=8KB contiguous bytes per partition line.
  - fp16 operands use power-of-2 prescales (M*1024, B*256) to stay
    clear of fp16 subnormals; the epilogue descales by 2^-18.
"""

import sys

if "/opt/trn_rl_repo" not in sys.path:
    sys.path.insert(0, "/opt/trn_rl_repo")

import numpy as np

B_, K, D = 64, 2048, 32400
NCORES = 8
DP = 32768
DL = DP // NCORES  # 4096
NDT = DL // 128    # 32 d-tiles
NQ = 2             # k-halves
KW = K // NQ       # 1024 k per half
NKB = K // 128     # 16 k-blocks
MCH = 4            # d-tiles per M DMA chunk (1MB per component)

SCALE_M = 1024.0
SCALE_B = 256.0

_nc_cache = {}


def build_nc(n_cores=NCORES):
    import concourse.mybir as mybir
    from concourse import bacc
    import concourse.tile as tile
    from concourse.masks import make_identity
    from concourse.tile_rust import add_dep_helper

    fp = mybir.dt.float16
    f32 = mybir.dt.float32

    nc = bacc.Bacc(
        trn_type="TRN2",
        target_bir_lowering=False,
        debug=False,
        num_devices=n_cores,
    )
    # xa[p, dt*128+j] = S_A[dt*128+p, j]; S_A = (Xr|Xi) along j
    xa = nc.dram_tensor("xa", [128, DL], fp, kind="ExternalInput")
    xb = nc.dram_tensor("xb", [128, DL], fp, kind="ExternalInput")
    # mrh[p, (q*NDT+dt)*KW + kk] = Mr[q*KW+kk, lo+dt*128+p]
    mrh = nc.dram_tensor("mrh", [128, NQ * NDT * KW], fp, kind="ExternalInput")
    mih = nc.dram_tensor("mih", [128, NQ * NDT * KW], fp, kind="ExternalInput")
    # bnr[p, kb*DL + dd] = Br[kb*128+p, lo+dd] * SCALE_B
    bnr = nc.dram_tensor("bnr", [128, NKB * DL], fp, kind="ExternalInput")
    bni = nc.dram_tensor("bni", [128, NKB * DL], fp, kind="ExternalInput")
    out = nc.dram_tensor("out", [128, DL], f32, kind="ExternalOutput")

    with tile.TileContext(nc) as tc:
        with (
            tc.tile_pool(name="sb", bufs=1) as sb,
            tc.tile_pool(name="sbx", bufs=1) as sbx,
            tc.tile_pool(name="ps", bufs=1, space="PSUM") as ps,
            tc.tile_pool(name="dram", bufs=1, space="DRAM") as dram,
        ):
            ident = sbx.tile([128, 128], f32, tag="ident")
            make_identity(nc, ident)

            xa_s = sbx.tile([128, DL], fp, tag="xa_s", name="xa_s")
            nc.sync.dma_start(out=xa_s, in_=xa.ap())
            xb_s = sbx.tile([128, DL], fp, tag="xb_s", name="xb_s")
            nc.sync.dma_start(out=xb_s, in_=xb.ap())

            # AR payload layout is the SBUF-native [128, KW] (the AllReduce
            # is elementwise, so any consistent layout works) -- this keeps
            # the arin write and arout read fully contiguous per partition.
            arin = []
            arout = []
            for q in range(NQ):
                ai = dram.tile([128, KW], fp, tag=f"arin{q}", name=f"arin{q}")
                ao = dram.tile(
                    [128, KW], fp, tag=f"arout{q}", name=f"arout{q}",
                    addr_space="Shared",
                )
                arin.append(ai)
                arout.append(ao)

            # ---------------- mm1: two k-halves ----------------
            scalar_dmas = []  # for explicit ordering on the scalar queue
            for q in range(NQ):
                acc0 = ps.tile([128, 512], f32, tag=f"p{2 * q}", name="acc0", bufs=1)
                acc1 = ps.tile([128, 512], f32, tag=f"p{2 * q + 1}", name="acc1", bufs=1)
                for g in range(NDT // MCH):
                    base = (q * NDT + g * MCH) * KW
                    mr_t = sb.tile([128, MCH * KW], fp, tag="mr", name="mr", bufs=4)
                    nc.sync.dma_start(out=mr_t, in_=mrh[:, base : base + MCH * KW])
                    mi_t = sb.tile([128, MCH * KW], fp, tag="mi", name="mi", bufs=4)
                    nc.sync.dma_start(out=mi_t, in_=mih[:, base : base + MCH * KW])
                    for t in range(MCH):
                        dt = g * MCH + t
                        st, sp = dt == 0, dt == NDT - 1
                        xs_a = xa_s[:, dt * 128 : (dt + 1) * 128]
                        xs_b = xb_s[:, dt * 128 : (dt + 1) * 128]
                        ts = t * KW
                        nc.tensor.matmul(acc0, lhsT=xs_a, rhs=mr_t[:, ts : ts + 512], start=st, stop=False)
                        nc.tensor.matmul(acc1, lhsT=xs_a, rhs=mr_t[:, ts + 512 : ts + 1024], start=st, stop=False)
                        nc.tensor.matmul(acc0, lhsT=xs_b, rhs=mi_t[:, ts : ts + 512], start=False, stop=sp)
                        nc.tensor.matmul(acc1, lhsT=xs_b, rhs=mi_t[:, ts + 512 : ts + 1024], start=False, stop=sp)

                # Evacuate Y[j, half], transpose to [k, j], ship to AR.
                pscr = sb.tile([128, KW], f32, tag="pscr", name="pscr", bufs=1)
                nc.vector.tensor_copy(pscr[:, 0:512], acc0)
                nc.scalar.copy(pscr[:, 512:1024], acc1)
                yt_sb = sb.tile([128, KW], fp, tag="yt_sb", name="yt_sb", bufs=1)
                for b in range(KW // 128):
                    tp = ps.tile([128, 128], f32, tag=f"p{4 + (b % 2)}", name="tp", bufs=1)
                    nc.tensor.transpose(tp, pscr[:, b * 128 : (b + 1) * 128], ident)
                    nc.vector.tensor_copy(yt_sb[:, b * 128 : (b + 1) * 128], tp)
                w_inst = nc.scalar.dma_start(out=arin[q], in_=yt_sb)
                scalar_dmas.append(w_inst)
                nc.gpsimd.collective_compute(
                    "AllReduce",
                    mybir.AluOpType.add,
                    ins=[arin[q].opt()],
                    outs=[arout[q].opt()],
                    replica_groups=[list(range(n_cores))],
                )

            # ---------------- mm2 stationaries from AR outputs ----------------
            ytA = []
            ytB = []
            for q in range(NQ):
                a_t = sbx.tile([128, KW], fp, tag=f"ytA{q}", name=f"ytA{q}")
                r_inst = nc.scalar.dma_start(out=a_t, in_=arout[q])
                scalar_dmas.append(r_inst)
                b_t = sbx.tile([128, KW], fp, tag=f"ytB{q}", name=f"ytB{q}")
                for b in range(KW // 128):
                    o = b * 128
                    nc.vector.tensor_scalar_mul(b_t[:, o : o + 64], a_t[:, o + 64 : o + 128], -1.0)
                    nc.vector.tensor_copy(b_t[:, o + 64 : o + 128], a_t[:, o : o + 64])
                ytA.append(a_t)
                ytB.append(b_t)

            # Pin the scalar-queue order: every arout read sits after every
            # arin write, and the queue follows emission order. Without this
            # the scheduler can park an AR-output read (waiting on AR k) in
            # front of a later AR-input write, stalling the next AR.
            from_i = None
            for inst in scalar_dmas:
                if from_i is not None:
                    add_dep_helper(inst.ins, from_i.ins, sync=False,
                                   reason="scalar DMA queue order")
                from_i = inst

            # ---------------- mm2: k-outer, 8 PSUM banks ----------------
            pos = []
            for i in range(8):
                po = ps.tile([128, 512], f32, tag=f"p{i}", name=f"po{i}", bufs=1)
                pos.append(po)
            for kb in range(NKB):
                q, b = kb // (KW // 128), kb % (KW // 128)
                br_t = sb.tile([128, DL], fp, tag="br", name="br", bufs=5)
                nc.sync.dma_start(out=br_t, in_=bnr[:, kb * DL : (kb + 1) * DL])
                bi_t = sb.tile([128, DL], fp, tag="bi", name="bi", bufs=5)
                nc.sync.dma_start(out=bi_t, in_=bni[:, kb * DL : (kb + 1) * DL])
                ya = ytA[q][:, b * 128 : (b + 1) * 128]
                yb = ytB[q][:, b * 128 : (b + 1) * 128]
                st, sp = kb == 0, kb == NKB - 1
                for i in range(8):
                    nc.tensor.matmul(pos[i], lhsT=ya, rhs=br_t[:, i * 512 : (i + 1) * 512], start=st, stop=False)
                for i in range(8):
                    nc.tensor.matmul(pos[i], lhsT=yb, rhs=bi_t[:, i * 512 : (i + 1) * 512], start=False, stop=sp)

            # Descale by 1/(SCALE_M*SCALE_B) happens on the host during
            # output assembly; here only evacuate PSUM -> SBUF.
            osb = sb.tile([128, DL], f32, tag="osb", name="osb", bufs=1)
            for r in range(4):
                lo = r * 1024
                nc.vector.tensor_copy(osb[:, lo : lo + 512], pos[2 * r])
                nc.vector.tensor_copy(osb[:, lo + 512 : lo + 1024], pos[2 * r + 1])
                nc.sync.dma_start(out=out[:, lo : lo + 1024], in_=osb[:, lo : lo + 1024])

    nc.compile()
    return nc


def _get_nc(n_cores=NCORES):
    if n_cores not in _nc_cache:
        _nc_cache[n_cores] = build_nc(n_cores)
    return _nc_cache[n_cores]


def _prep_in_maps(X_re, X_im, bases_re, bases_im, weight_re, weight_im):
    cdt = np.float16
    f32 = np.float32
    X_re = np.asarray(X_re, f32)
    X_im = np.asarray(X_im, f32)
    bases_re = np.asarray(bases_re, f32)
    bases_im = np.asarray(bases_im, f32)
    wr = np.asarray(weight_re, f32)[:, None]
    wi = np.asarray(weight_im, f32)[:, None]

    # M = diag(w) @ conj(B): Mr = wr*Br + wi*Bi ; Mi = wi*Br - wr*Bi
    mr = (wr * bases_re + wi * bases_im) * f32(SCALE_M)
    mi = (wi * bases_re - wr * bases_im) * f32(SCALE_M)
    bsr = bases_re * f32(SCALE_B)
    bsi = bases_im * f32(SCALE_B)

    in_maps = []
    for c in range(NCORES):
        lo = c * DL
        hi = min((c + 1) * DL, D)
        n = hi - lo

        xat = np.zeros((DL, 128), f32)
        xbt = np.zeros((DL, 128), f32)
        if n > 0:
            xat[:n, 0:64] = X_re[:, lo:hi].T
            xat[:n, 64:128] = X_im[:, lo:hi].T
            xbt[:n, 0:64] = -X_im[:, lo:hi].T
            xbt[:n, 64:128] = X_re[:, lo:hi].T
        # [DL,128] -> [128, DL] with xa[p, dt*128+j] = xat[dt*128+p, j]
        xa = xat.reshape(NDT, 128, 128).transpose(1, 0, 2).reshape(128, DL).astype(cdt)
        xb = xbt.reshape(NDT, 128, 128).transpose(1, 0, 2).reshape(128, DL).astype(cdt)

        def m_layout(m):
            mp = np.zeros((K, DL), f32)
            if n > 0:
                mp[:, :n] = m[:, lo:hi]
            # mrh[p, (q*NDT+dt)*KW + kk] = mp[q*KW+kk, dt*128+p]
            t = mp.reshape(NQ, KW, NDT, 128)
            return t.transpose(3, 0, 2, 1).reshape(128, NQ * NDT * KW).astype(cdt)

        def b_layout(bm):
            bp = np.zeros((K, DL), f32)
            if n > 0:
                bp[:, :n] = bm[:, lo:hi]
            # bnr[p, kb*DL + dd] = bp[kb*128+p, dd]
            t = bp.reshape(NKB, 128, DL)
            return t.transpose(1, 0, 2).reshape(128, NKB * DL).astype(cdt)

        in_maps.append({
            "xa": xa,
            "xb": xb,
            "mrh": m_layout(mr),
            "mih": m_layout(mi),
            "bnr": b_layout(bsr),
            "bni": b_layout(bsi),
        })
    return in_maps


def run(inputs, trace=False, trace_kwargs=None):
    """Returns (full complex64 output [64, 32400], BassKernelResults)."""
    from concourse.bass_utils import run_bass_kernel_spmd

    in_maps = _prep_in_maps(**inputs)
    nc = _get_nc()
    res = run_bass_kernel_spmd(
        nc,
        in_maps,
        core_ids=list(range(NCORES)),
        trace=trace,
        **(trace_kwargs or {}),
    )
    dsc = np.float32(1.0 / (SCALE_M * SCALE_B))
    parts = []
    for c in range(NCORES):
        o = res.results[c]["out"]
        parts.append(o[0:64, :] + 1j * o[64:128, :].astype(np.complex64))
    full = (np.concatenate(parts, axis=1)[:, :D] * dsc).astype(np.complex64)
    return full, res


def kernel(**inputs) -> np.ndarray:
    out, _ = run(inputs, trace=False)
    return out


# revision 18
# speedup vs baseline: 1.0801x; 1.0084x over previous
"""ComplexDenseSO2 Trainium2 kernel (v3).

Computes out = (X @ conj(B)^T * w) @ B for complex X [64, 32400],
B [2048, 32400], w [2048], given as separate re/im fp32 planes.

Strategy (tensor-parallel over D across 8 cores):
  - Fold w into the first-matmul operand on the host:
    M = diag(w) @ conj(B), so mm1 output IS Y = X @ M^T.
  - Pad D 32400 -> 32768; core c owns d-slice [c*4096, (c+1)*4096).
  - mm1 merges re/im into ONE PSUM accumulator using two stationary
    variants S_A = (Xr|Xi), S_B = (-Xi|Xr): rows 0:64 = Yr, 64:128 = Yi.
    k is processed in 2 HALVES of 1024 (2 PSUM banks each) so each
    half's 256KB AllReduce overlaps the other half's compute and the
    start of mm2.
  - Per half: PSUM -> SBUF f32, PE-transpose to [k, j], cast fp16, one
    contiguous 256KB DMA to a [128, 1024]-layout DRAM bounce (the AR is
    elementwise, so the SBUF-native layout avoids small descriptors),
    AllReduce(add) over the 8 cores.
  - mm2: k-outer with all 8 PSUM banks holding the full [128, 4096]
    f32 output; stationaries ytA = Y^T, ytB = (-Yi^T|Yr^T) built from
    the AR outputs; rhs B tiles streamed as 1MB fully-contiguous DMAs.
  - DMA engine split: input streams (X, M, B) issue on nc.sync;
    AR-coupled transfers (arin writes, arout reads) issue on
    nc.scalar with explicit ordering deps so the scheduler cannot
    park an AR-output read in front of a later AR-input write.
  - Host pre-lays out every tensor so each big DMA is >=1MB with
    >=8KB contiguous bytes per partition line.
  - fp16 operands use power-of-2 prescales (M*1024, B*256) to stay
    clear of fp16 subnormals; the epilogue descales by 2^-18.
"""

import sys

if "/opt/trn_rl_repo" not in sys.path:
    sys.path.insert(0, "/opt/trn_rl_repo")

import numpy as np

B_, K, D = 64, 2048, 32400
NCORES = 8
DP = 32768
DL = DP // NCORES  # 4096
NDT = DL // 128    # 32 d-tiles
NQ = 2             # k-halves
KW = K // NQ       # 1024 k per half
NKB = K // 128     # 16 k-blocks
MCH = 4            # d-tiles per M DMA chunk (1MB per component)

SCALE_M = 1024.0
SCALE_B = 256.0

_nc_cache = {}


def build_nc(n_cores=NCORES):
    import concourse.mybir as mybir
    from concourse import bacc
    import concourse.tile as tile
    from concourse.masks import make_identity
    from concourse.tile_rust import add_dep_helper

    fp = mybir.dt.float16
    f32 = mybir.dt.float32

    nc = bacc.Bacc(
        trn_type="TRN2",
        target_bir_lowering=False,
        debug=False,
        num_devices=n_cores,
    )
    # xa[p, dt*128+j] = S_A[dt*128+p, j]; S_A = (Xr|Xi) along j
    xa = nc.dram_tensor("xa", [128, DL], fp, kind="ExternalInput")
    xb = nc.dram_tensor("xb", [128, DL], fp, kind="ExternalInput")
    # mrh[p, (q*NDT+dt)*KW + kk] = Mr[q*KW+kk, lo+dt*128+p]
    mrh = nc.dram_tensor("mrh", [128, NQ * NDT * KW], fp, kind="ExternalInput")
    mih = nc.dram_tensor("mih", [128, NQ * NDT * KW], fp, kind="ExternalInput")
    # bnr[p, kb*DL + dd] = Br[kb*128+p, lo+dd] * SCALE_B
    bnr = nc.dram_tensor("bnr", [128, NKB * DL], fp, kind="ExternalInput")
    bni = nc.dram_tensor("bni", [128, NKB * DL], fp, kind="ExternalInput")
    out = nc.dram_tensor("out", [128, DL], f32, kind="ExternalOutput")

    with tile.TileContext(nc) as tc:
        with (
            tc.tile_pool(name="sb", bufs=1) as sb,
            tc.tile_pool(name="sbx", bufs=1) as sbx,
            tc.tile_pool(name="ps", bufs=1, space="PSUM") as ps,
            tc.tile_pool(name="dram", bufs=1, space="DRAM") as dram,
        ):
            ident = sbx.tile([128, 128], f32, tag="ident")
            make_identity(nc, ident)

            xa_s = sbx.tile([128, DL], fp, tag="xa_s", name="xa_s")
            nc.sync.dma_start(out=xa_s, in_=xa.ap())
            xb_s = sbx.tile([128, DL], fp, tag="xb_s", name="xb_s")
            nc.sync.dma_start(out=xb_s, in_=xb.ap())

            # AR payload layout is the SBUF-native [128, KW] (the AllReduce
            # is elementwise, so any consistent layout works) -- this keeps
            # the arin write and arout read fully contiguous per partition.
            arin = []
            arout = []
            for q in range(NQ):
                ai = dram.tile([128, KW], fp, tag=f"arin{q}", name=f"arin{q}")
                ao = dram.tile(
                    [128, KW], fp, tag=f"arout{q}", name=f"arout{q}",
                    addr_space="Shared",
                )
                arin.append(ai)
                arout.append(ao)

            # ---------------- mm1: two k-halves ----------------
            scalar_dmas = []  # for explicit ordering on the scalar queue
            for q in range(NQ):
                acc0 = ps.tile([128, 512], f32, tag=f"p{2 * q}", name="acc0", bufs=1)
                acc1 = ps.tile([128, 512], f32, tag=f"p{2 * q + 1}", name="acc1", bufs=1)
                # Final 4 d-tiles split into 2-tile chunks so the tail
                # matmuls (and thus the AR trigger) chase the stream closer.
                chunks = [MCH] * (NDT // MCH - 1) + [MCH // 2, MCH // 2]
                dt0 = 0
                for csz in chunks:
                    base = (q * NDT + dt0) * KW
                    mr_t = sb.tile([128, csz * KW], fp, tag="mr", name="mr", bufs=4)
                    nc.sync.dma_start(out=mr_t, in_=mrh[:, base : base + csz * KW])
                    mi_t = sb.tile([128, csz * KW], fp, tag="mi", name="mi", bufs=4)
                    nc.sync.dma_start(out=mi_t, in_=mih[:, base : base + csz * KW])
                    for t in range(csz):
                        dt = dt0 + t
                        st, sp = dt == 0, dt == NDT - 1
                        xs_a = xa_s[:, dt * 128 : (dt + 1) * 128]
                        xs_b = xb_s[:, dt * 128 : (dt + 1) * 128]
                        ts = t * KW
                        nc.tensor.matmul(acc0, lhsT=xs_a, rhs=mr_t[:, ts : ts + 512], start=st, stop=False)
                        nc.tensor.matmul(acc1, lhsT=xs_a, rhs=mr_t[:, ts + 512 : ts + 1024], start=st, stop=False)
                        nc.tensor.matmul(acc0, lhsT=xs_b, rhs=mi_t[:, ts : ts + 512], start=False, stop=sp)
                        nc.tensor.matmul(acc1, lhsT=xs_b, rhs=mi_t[:, ts + 512 : ts + 1024], start=False, stop=sp)
                    dt0 += csz

                # Evacuate Y[j, half], transpose to [k, j], ship to AR.
                pscr = sb.tile([128, KW], f32, tag="pscr", name="pscr", bufs=1)
                nc.vector.tensor_copy(pscr[:, 0:512], acc0)
                nc.scalar.copy(pscr[:, 512:1024], acc1)
                yt_sb = sb.tile([128, KW], fp, tag="yt_sb", name="yt_sb", bufs=1)
                for b in range(KW // 128):
                    tp = ps.tile([128, 128], f32, tag=f"p{4 + (b % 2)}", name="tp", bufs=1)
                    nc.tensor.transpose(tp, pscr[:, b * 128 : (b + 1) * 128], ident)
                    nc.vector.tensor_copy(yt_sb[:, b * 128 : (b + 1) * 128], tp)
                w_inst = nc.scalar.dma_start(out=arin[q], in_=yt_sb)
                scalar_dmas.append(w_inst)
                nc.gpsimd.collective_compute(
                    "AllReduce",
                    mybir.AluOpType.add,
                    ins=[arin[q].opt()],
                    outs=[arout[q].opt()],
                    replica_groups=[list(range(n_cores))],
                )

            # ---------------- mm2 stationaries from AR outputs ----------------
            ytA = []
            ytB = []
            for q in range(NQ):
                a_t = sbx.tile([128, KW], fp, tag=f"ytA{q}", name=f"ytA{q}")
                r_inst = nc.scalar.dma_start(out=a_t, in_=arout[q])
                scalar_dmas.append(r_inst)
                b_t = sbx.tile([128, KW], fp, tag=f"ytB{q}", name=f"ytB{q}")
                for b in range(KW // 128):
                    o = b * 128
                    nc.vector.tensor_scalar_mul(b_t[:, o : o + 64], a_t[:, o + 64 : o + 128], -1.0)
                    nc.vector.tensor_copy(b_t[:, o + 64 : o + 128], a_t[:, o : o + 64])
                ytA.append(a_t)
                ytB.append(b_t)

            # Pin the scalar-queue order: every arout read sits after every
            # arin write, and the queue follows emission order. Without this
            # the scheduler can park an AR-output read (waiting on AR k) in
            # front of a later AR-input write, stalling the next AR.
            from_i = None
            for inst in scalar_dmas:
                if from_i is not None:
                    add_dep_helper(inst.ins, from_i.ins, sync=False,
                                   reason="scalar DMA queue order")
                from_i = inst

            # ---------------- mm2: k-outer, 8 PSUM banks ----------------
            pos = []
            for i in range(8):
                po = ps.tile([128, 512], f32, tag=f"p{i}", name=f"po{i}", bufs=1)
                pos.append(po)
            for kb in range(NKB):
                q, b = kb // (KW // 128), kb % (KW // 128)
                br_t = sb.tile([128, DL], fp, tag="br", name="br", bufs=5)
                nc.sync.dma_start(out=br_t, in_=bnr[:, kb * DL : (kb + 1) * DL])
                bi_t = sb.tile([128, DL], fp, tag="bi", name="bi", bufs=5)
                nc.sync.dma_start(out=bi_t, in_=bni[:, kb * DL : (kb + 1) * DL])
                ya = ytA[q][:, b * 128 : (b + 1) * 128]
                yb = ytB[q][:, b * 128 : (b + 1) * 128]
                st, sp = kb == 0, kb == NKB - 1
                for i in range(8):
                    nc.tensor.matmul(pos[i], lhsT=ya, rhs=br_t[:, i * 512 : (i + 1) * 512], start=st, stop=False)
                for i in range(8):
                    nc.tensor.matmul(pos[i], lhsT=yb, rhs=bi_t[:, i * 512 : (i + 1) * 512], start=False, stop=sp)

            # Descale by 1/(SCALE_M*SCALE_B) happens on the host during
            # output assembly; here only evacuate PSUM -> SBUF.
            osb = sb.tile([128, DL], f32, tag="osb", name="osb", bufs=1)
            for r in range(4):
                lo = r * 1024
                nc.vector.tensor_copy(osb[:, lo : lo + 512], pos[2 * r])
                nc.vector.tensor_copy(osb[:, lo + 512 : lo + 1024], pos[2 * r + 1])
                nc.sync.dma_start(out=out[:, lo : lo + 1024], in_=osb[:, lo : lo + 1024])

    nc.compile()
    return nc


def _get_nc(n_cores=NCORES):
    if n_cores not in _nc_cache:
        _nc_cache[n_cores] = build_nc(n_cores)
    return _nc_cache[n_cores]


def _prep_in_maps(X_re, X_im, bases_re, bases_im, weight_re, weight_im):
    cdt = np.float16
    f32 = np.float32
    X_re = np.asarray(X_re, f32)
    X_im = np.asarray(X_im, f32)
    bases_re = np.asarray(bases_re, f32)
    bases_im = np.asarray(bases_im, f32)
    wr = np.asarray(weight_re, f32)[:, None]
    wi = np.asarray(weight_im, f32)[:, None]

    # M = diag(w) @ conj(B): Mr = wr*Br + wi*Bi ; Mi = wi*Br - wr*Bi
    mr = (wr * bases_re + wi * bases_im) * f32(SCALE_M)
    mi = (wi * bases_re - wr * bases_im) * f32(SCALE_M)
    bsr = bases_re * f32(SCALE_B)
    bsi = bases_im * f32(SCALE_B)

    in_maps = []
    for c in range(NCORES):
        lo = c * DL
        hi = min((c + 1) * DL, D)
        n = hi - lo

        xat = np.zeros((DL, 128), f32)
        xbt = np.zeros((DL, 128), f32)
        if n > 0:
            xat[:n, 0:64] = X_re[:, lo:hi].T
            xat[:n, 64:128] = X_im[:, lo:hi].T
            xbt[:n, 0:64] = -X_im[:, lo:hi].T
            xbt[:n, 64:128] = X_re[:, lo:hi].T
        # [DL,128] -> [128, DL] with xa[p, dt*128+j] = xat[dt*128+p, j]
        xa = xat.reshape(NDT, 128, 128).transpose(1, 0, 2).reshape(128, DL).astype(cdt)
        xb = xbt.reshape(NDT, 128, 128).transpose(1, 0, 2).reshape(128, DL).astype(cdt)

        def m_layout(m):
            mp = np.zeros((K, DL), f32)
            if n > 0:
                mp[:, :n] = m[:, lo:hi]
            # mrh[p, (q*NDT+dt)*KW + kk] = mp[q*KW+kk, dt*128+p]
            t = mp.reshape(NQ, KW, NDT, 128)
            return t.transpose(3, 0, 2, 1).reshape(128, NQ * NDT * KW).astype(cdt)

        def b_layout(bm):
            bp = np.zeros((K, DL), f32)
            if n > 0:
                bp[:, :n] = bm[:, lo:hi]
            # bnr[p, kb*DL + dd] = bp[kb*128+p, dd]
            t = bp.reshape(NKB, 128, DL)
            return t.transpose(1, 0, 2).reshape(128, NKB * DL).astype(cdt)

        in_maps.append({
            "xa": xa,
            "xb": xb,
            "mrh": m_layout(mr),
            "mih": m_layout(mi),
            "bnr": b_layout(bsr),
            "bni": b_layout(bsi),
        })
    return in_maps


def run(inputs, trace=False, trace_kwargs=None):
    """Returns (full complex64 output [64, 32400], BassKernelResults)."""
    from concourse.bass_utils import run_bass_kernel_spmd

    in_maps = _prep_in_maps(**inputs)
    nc = _get_nc()
    res = run_bass_kernel_spmd(
        nc,
        in_maps,
        core_ids=list(range(NCORES)),
        trace=trace,
        **(trace_kwargs or {}),
    )
    dsc = np.float32(1.0 / (SCALE_M * SCALE_B))
    parts = []
    for c in range(NCORES):
        o = res.results[c]["out"]
        parts.append(o[0:64, :] + 1j * o[64:128, :].astype(np.complex64))
    full = (np.concatenate(parts, axis=1)[:, :D] * dsc).astype(np.complex64)
    return full, res


def kernel(**inputs) -> np.ndarray:
    out, _ = run(inputs, trace=False)
    return out
